# revision 16
# baseline (speedup 1.0000x reference)
"""DCGRU cell (dense diffusion graph conv GRU) on 8 Trainium2 NeuronCores.

Strategy (data-parallel over batch, 4 batch elements per core):
  - supports are pre-transposed on host (A^T, contiguous) so the tensor
    engine can contract over the node dimension m with m on partitions.
  - node-feature matrices that feed the final linear are kept in
    TRANSPOSED form h^T [features, nodes]; hop outputs are produced
    transposed directly:  Y^T = matmul(lhsT=x_chunk [m,34], rhs=A^T [m,n]).
  - the final linear is 5 accumulating matmuls, one per 34-row feature
    block (weights sliced host-side to match).
  - matmuls run as float32r (fp32 storage, single-pass reduced-precision
    PE mode, 1 cycle/row at free-dim 512; PSUM dst must start at
    partition 0); transposes stay exact fp32.
  - gate math is done in transposed space; the output is produced as
    out^T [H, N] and un-transposed on host.
"""

import hashlib
import inspect

import numpy as np

import concourse.bacc as bacc
import concourse.mybir as mybir
import concourse.tile as tile
import concourse.bass as bass
from concourse.bass_utils import run_bass_kernel_spmd
from concourse.masks import make_identity

B, N, CIN, H, S, K = 32, 1024, 2, 32, 2, 2
NCORES = 8
BL = B // NCORES          # batch elements per core
CC = CIN + H              # 34 concat features
NT = N // 128             # 8 node chunks of 128
F32 = mybir.dt.float32
F32R = mybir.dt.float32r
AF = mybir.ActivationFunctionType
OP = mybir.AluOpType

_CACHE = {}
LAST_RESULTS = None

W_NAMES = ["w0", "w1a", "w1b", "w2a", "w2b"]


def _build():
    # Version-tag the output tensor name with a hash of this function's
    # source: the neuron compile cache keys on HLO (which embeds tensor
    # names but not the BIR), so kernel-internal changes need a name
    # change to invalidate stale NEFFs.
    vtag = hashlib.md5(inspect.getsource(_build).encode()).hexdigest()[:8]
    _CACHE["out_name"] = f"outT_{vtag}"
    nc = bacc.Bacc("TRN2", target_bir_lowering=False, debug=False,
                   num_devices=NCORES)

    inp = nc.dram_tensor("inp", [BL, N, CIN], F32R, kind="ExternalInput")
    inpT = nc.dram_tensor("inpT", [BL, CIN, N], F32R, kind="ExternalInput")
    st = nc.dram_tensor("st", [BL, N, H], F32R, kind="ExternalInput")
    stT = nc.dram_tensor("stT", [BL, H, N], F32R, kind="ExternalInput")
    supT = nc.dram_tensor("supT", [BL, S, N, N], F32R, kind="ExternalInput")
    wru_d = [nc.dram_tensor(f"wru_{k}", [CC, 2 * H], F32R,
                            kind="ExternalInput") for k in W_NAMES]
    wc_d = [nc.dram_tensor(f"wc_{k}", [CC, H], F32R,
                           kind="ExternalInput") for k in W_NAMES]
    bru = nc.dram_tensor("bru", [2 * H], F32, kind="ExternalInput")
    bc = nc.dram_tensor("bc", [H], F32, kind="ExternalInput")
    outT = nc.dram_tensor(_CACHE["out_name"], [BL, H, N], F32,
                          kind="ExternalOutput")

    with tile.TileContext(nc) as tc:
        with (
            tc.tile_pool(name="const", bufs=1) as constp,
            tc.tile_pool(name="at", bufs=2 * NT) as atp,
            tc.tile_pool(name="x", bufs=2) as xp,
            tc.tile_pool(name="h", bufs=2) as hp,
            tc.tile_pool(name="act", bufs=1) as actp,
            tc.tile_pool(name="hop_ps", bufs=4, space=bass.MemorySpace.PSUM) as hop_ps,
            tc.tile_pool(name="tr_ps", bufs=2, space=bass.MemorySpace.PSUM) as tr_ps,
            tc.tile_pool(name="lin_ps", bufs=2, space=bass.MemorySpace.PSUM) as lin_ps,
        ):
            ident = constp.tile([128, 128], F32)
            make_identity(nc, ident[:])

            wru_s = [constp.tile([CC, 2 * H], F32R, tag=f"wru{k}",
                                 name=f"wru_s{k}") for k in W_NAMES]
            wc_s = [constp.tile([CC, H], F32R, tag=f"wc{k}",
                                name=f"wc_s{k}") for k in W_NAMES]
            brus = constp.tile([2 * H, 1], F32, tag="bru")
            bcs = constp.tile([H, 1], F32, tag="bc")
            for d, s_ in zip(wru_d, wru_s):
                nc.sync.dma_start(s_[:], d[:])
            for d, s_ in zip(wc_d, wc_s):
                nc.sync.dma_start(s_[:], d[:])
            nc.sync.dma_start(brus[:], bru[:].unsqueeze(1))
            nc.sync.dma_start(bcs[:], bc[:].unsqueeze(1))

            def hop_stage(x0, x1_, at0, at1, ps):
                """One diffusion hop for both supports.
                x0/x1_: [128, NT, CC] natural lhsT tiles
                at0/at1: lists of NT chunk tiles [128, N]
                ps: dict (s, j) -> PSUM tile [CC, 512] = Y_s^T half."""
                for t in range(NT):
                    for s_, xt in ((0, x0), (1, x1_)):
                        lt = xt[:, t, :]
                        at = at0[t] if s_ == 0 else at1[t]
                        for j in range(2):
                            nc.tensor.matmul(
                                ps[s_, j][:],
                                lt, at[:, j * 512:(j + 1) * 512],
                                start=(t == 0), stop=(t == NT - 1),
                                skip_group_check=True)

            def to_natural(hT, nat):
                """Transpose hT[0:CC, :] ([CC, N]) into natural
                [128, NT, CC] tile nat."""
                ps = tr_ps.tile([128, NT * CC], F32, tag="tr")
                for t in range(NT):
                    nc.tensor.transpose(
                        ps[:, t * CC:(t + 1) * CC],
                        hT[0:CC, t * 128:(t + 1) * 128].bitcast(F32),
                        ident[0:CC, 0:CC],
                    )
                nc.vector.tensor_copy(
                    nat[:, :, :],
                    ps[:].rearrange("p (t c) -> p t c", c=CC))

            for b in range(BL):
                # natural x1 = [inputs | states]
                x1 = xp.tile([128, NT, CC], F32R, tag="x1")
                nc.sync.dma_start(
                    x1[:, :, 0:CIN],
                    inp[b].rearrange("(t p) c -> p t c", p=128))
                nc.sync.dma_start(
                    x1[:, :, CIN:CC],
                    st[b].rearrange("(t p) c -> p t c", p=128))

                # A^T chunk tiles (one per m-block, 512 KB each) so hop
                # matmuls start as soon as chunk t=0 lands
                sup_r0 = supT[b, 0].rearrange("(t p) m -> p t m", p=128)
                sup_r1 = supT[b, 1].rearrange("(t p) m -> p t m", p=128)
                at0, at1 = [], []
                for t in range(NT):
                    a0 = atp.tile([128, N], F32R, tag="at0", name=f"at0_{t}")
                    a1 = atp.tile([128, N], F32R, tag="at1", name=f"at1_{t}")
                    nc.sync.dma_start(a0[:], sup_r0[:, t, :])
                    nc.sync.dma_start(a1[:], sup_r1[:, t, :])
                    at0.append(a0)
                    at1.append(a1)

                # states^T for gate & r*state
                stT_sb = actp.tile([H, N], F32R, tag="stT")
                nc.sync.dma_start(stT_sb[:], stT[b])

                ruT = actp.tile([2 * H, N], F32, tag="ruT")
                cT = actp.tile([H, N], F32, tag="cT")

                for conv in range(2):
                    # ---- hT0 = x^T: rows 0:2 inputs^T, 2:34 (r*)states^T
                    hT0 = hp.tile([CC, N], F32R, tag="hT0")
                    nc.sync.dma_start(hT0[0:CIN, :], inpT[b])
                    if conv == 0:
                        nc.sync.dma_start(hT0[CIN:CC, :], stT[b])
                        xn = x1
                    else:
                        # r^T * states^T (computed base-0, partition-shifted
                        # into hT0 rows 2:34 via SBUF->SBUF DMA)
                        rstT = actp.tile([H, N], F32R, tag="scr")
                        nc.vector.tensor_tensor(
                            out=rstT[:], in0=ruT[0:H, :],
                            in1=stT_sb[:].bitcast(F32), op=OP.mult)
                        nc.sync.dma_start(hT0[CIN:CC, :], rstT[:])
                        # natural x2: transpose r^T, then r*state
                        psr = tr_ps.tile([128, NT * H], F32, tag="tr")
                        for t in range(NT):
                            nc.tensor.transpose(
                                psr[:, t * H:(t + 1) * H],
                                ruT[0:H, t * 128:(t + 1) * 128],
                                ident[0:H, 0:H])
                        x2 = xp.tile([128, NT, CC], F32R, tag="x2")
                        nc.sync.dma_start(
                            x2[:, :, 0:CIN],
                            inp[b].rearrange("(t p) c -> p t c", p=128))
                        nc.vector.tensor_tensor(
                            out=x2[:, :, CIN:CC],
                            in0=psr[:].rearrange("p (t c) -> p t c", c=H),
                            in1=x1[:, :, CIN:CC].bitcast(F32), op=OP.mult)
                        xn = x2

                    # ---- hop 1 (both supports) ----
                    h1ps = {(s_, j): hop_ps.tile([CC, 512], F32, tag="hop",
                                                 name=f"h1ps{s_}{j}")
                            for s_ in range(2) for j in range(2)}
                    hop_stage(xn, xn, at0, at1, h1ps)
                    hT1a = hp.tile([CC, N], F32R, tag="hT1a")
                    hT1b = hp.tile([CC, N], F32R, tag="hT1b")
                    for j in range(2):
                        sl = slice(j * 512, (j + 1) * 512)
                        nc.vector.tensor_copy(hT1a[:, sl], h1ps[0, j][:])
                        nc.vector.tensor_copy(hT1b[:, sl], h1ps[1, j][:])

                    # ---- naturals of hop-1 outputs ----
                    y0 = xp.tile([128, NT, CC], F32R, tag="y0")
                    y1 = xp.tile([128, NT, CC], F32R, tag="y1")
                    to_natural(hT1a, y0)
                    to_natural(hT1b, y1)

                    # ---- hop 2 ----
                    h2ps = {(s_, j): hop_ps.tile([CC, 512], F32, tag="hop",
                                                 name=f"h2ps{s_}{j}")
                            for s_ in range(2) for j in range(2)}
                    hop_stage(y0, y1, at0, at1, h2ps)
                    hT2a = hp.tile([CC, N], F32R, tag="hT2a")
                    hT2b = hp.tile([CC, N], F32R, tag="hT2b")
                    for j in range(2):
                        sl = slice(j * 512, (j + 1) * 512)
                        nc.vector.tensor_copy(hT2a[:, sl], h2ps[0, j][:])
                        nc.vector.tensor_copy(hT2b[:, sl], h2ps[1, j][:])

                    # ---- linear + activation ----
                    if conv == 0:
                        ws, Mo, bias, dest, func = \
                            wru_s, 2 * H, brus, ruT, AF.Sigmoid
                    else:
                        ws, Mo, bias, dest, func = wc_s, H, bcs, cT, AF.Tanh
                    rhs_tiles = [hT0, hT1a, hT1b, hT2a, hT2b]
                    for j in range(2):
                        pl = lin_ps.tile([2 * H, 512], F32, tag="lin")
                        sl = slice(j * 512, (j + 1) * 512)
                        for k in range(5):
                            nc.tensor.matmul(pl[0:Mo, :], ws[k][:],
                                             rhs_tiles[k][:, sl],
                                             start=(k == 0), stop=(k == 4))
                        nc.scalar.activation(dest[:, sl], pl[0:Mo, :],
                                             func, bias=bias[:])

                # ---- gate: out^T = u^T*(st^T - c^T) + c^T ----
                # u^T rebased to partition 0 via SBUF->SBUF DMA
                uT = actp.tile([H, N], F32, tag="uT")
                nc.sync.dma_start(uT[:], ruT[H:2 * H, :])
                oT = actp.tile([H, N], F32, tag="oT")
                nc.vector.tensor_tensor(out=oT[:], in0=stT_sb[:].bitcast(F32),
                                        in1=cT[:], op=OP.subtract)
                nc.vector.tensor_tensor(out=oT[:], in0=uT[:],
                                        in1=oT[:], op=OP.mult)
                nc.vector.tensor_tensor(out=oT[:], in0=oT[:],
                                        in1=cT[:], op=OP.add)
                nc.sync.dma_start(outT[b], oT[:])

    nc.compile()
    return nc


def _split_w(W):
    """Split W [170, cols] into the 5 feature blocks in h^T tile order:
    x, A0x, A1x, A0^2 x, A1^2 x."""
    blocks = [W[0:CC]]
    for fidx in (1, 3, 2, 4):
        blocks.append(W[fidx * CC:(fidx + 1) * CC])
    return [np.ascontiguousarray(w, np.float32) for w in blocks]


def kernel(inputs, supports, states, W_ru, b_ru, W_c, b_c, **_ignored):
    global LAST_RESULTS
    inputs = np.ascontiguousarray(np.asarray(inputs, np.float32))
    supports = np.asarray(supports, np.float32)
    states = np.ascontiguousarray(np.asarray(states, np.float32))
    W_ru = np.asarray(W_ru, np.float32)
    b_ru = np.ascontiguousarray(np.asarray(b_ru, np.float32))
    W_c = np.asarray(W_c, np.float32)
    b_c = np.ascontiguousarray(np.asarray(b_c, np.float32))

    supT = np.ascontiguousarray(np.swapaxes(supports, 2, 3))
    inpT = np.ascontiguousarray(np.swapaxes(inputs, 1, 2))
    stT = np.ascontiguousarray(np.swapaxes(states, 1, 2))
    wru_blocks = _split_w(W_ru)
    wc_blocks = _split_w(W_c)

    if "nc" not in _CACHE:
        _CACHE["nc"] = _build()
    nc = _CACHE["nc"]

    in_maps = []
    for i in range(NCORES):
        sl = slice(i * BL, (i + 1) * BL)
        m = {
            "inp": inputs[sl], "inpT": inpT[sl],
            "st": states[sl], "stT": stT[sl], "supT": supT[sl],
            "bru": b_ru, "bc": b_c,
        }
        for k, wb in zip(W_NAMES, wru_blocks):
            m[f"wru_{k}"] = wb
        for k, wb in zip(W_NAMES, wc_blocks):
            m[f"wc_{k}"] = wb
        in_maps.append(m)

    res = run_bass_kernel_spmd(nc, in_maps, list(range(NCORES)))
    LAST_RESULTS = res
    oname = _CACHE["out_name"]
    out = np.concatenate(
        [np.swapaxes(res.results[i][oname], 1, 2) for i in range(NCORES)],
        axis=0)
    return (out, out)


# revision 17
# speedup vs baseline: 1.0092x; 1.0092x over previous
"""DCGRU cell (dense diffusion graph conv GRU) on 8 Trainium2 NeuronCores.

Strategy (data-parallel over batch, 4 batch elements per core):
  - supports are pre-transposed on host (A^T, contiguous) so the tensor
    engine can contract over the node dimension m with m on partitions.
  - node-feature matrices that feed the final linear are kept in
    TRANSPOSED form h^T [features, nodes]; hop outputs are produced
    transposed directly:  Y^T = matmul(lhsT=x_chunk [m,34], rhs=A^T [m,n]).
  - the final linear is 5 accumulating matmuls, one per 34-row feature
    block (weights sliced host-side to match).
  - matmuls run as float32r (fp32 storage, single-pass reduced-precision
    PE mode, 1 cycle/row at free-dim 512; PSUM dst must start at
    partition 0); transposes stay exact fp32.
  - gate math is done in transposed space; the output is produced as
    out^T [H, N] and un-transposed on host.
"""

import hashlib
import inspect

import numpy as np

import concourse.bacc as bacc
import concourse.mybir as mybir
import concourse.tile as tile
import concourse.bass as bass
from concourse.bass_utils import run_bass_kernel_spmd
from concourse.masks import make_identity

B, N, CIN, H, S, K = 32, 1024, 2, 32, 2, 2
NCORES = 8
BL = B // NCORES          # batch elements per core
CC = CIN + H              # 34 concat features
NT = N // 128             # 8 node chunks of 128
F32 = mybir.dt.float32
F32R = mybir.dt.float32r
AF = mybir.ActivationFunctionType
OP = mybir.AluOpType

_CACHE = {}
LAST_RESULTS = None

W_NAMES = ["w0", "w1a", "w1b", "w2a", "w2b"]


def _build():
    # Version-tag the output tensor name with a hash of this function's
    # source: the neuron compile cache keys on HLO (which embeds tensor
    # names but not the BIR), so kernel-internal changes need a name
    # change to invalidate stale NEFFs.
    vtag = hashlib.md5(inspect.getsource(_build).encode()).hexdigest()[:8]
    _CACHE["out_name"] = f"outT_{vtag}"
    nc = bacc.Bacc("TRN2", target_bir_lowering=False, debug=False,
                   num_devices=NCORES)

    inp = nc.dram_tensor("inp", [BL, N, CIN], F32R, kind="ExternalInput")
    inpT = nc.dram_tensor("inpT", [BL, CIN, N], F32R, kind="ExternalInput")
    st = nc.dram_tensor("st", [BL, N, H], F32R, kind="ExternalInput")
    stT = nc.dram_tensor("stT", [BL, H, N], F32R, kind="ExternalInput")
    supT = nc.dram_tensor("supT", [BL, S, N, N], F32R, kind="ExternalInput")
    wru_d = [nc.dram_tensor(f"wru_{k}", [CC, 2 * H], F32R,
                            kind="ExternalInput") for k in W_NAMES]
    wc_d = [nc.dram_tensor(f"wc_{k}", [CC, H], F32R,
                           kind="ExternalInput") for k in W_NAMES]
    bru = nc.dram_tensor("bru", [2 * H], F32, kind="ExternalInput")
    bc = nc.dram_tensor("bc", [H], F32, kind="ExternalInput")
    outT = nc.dram_tensor(_CACHE["out_name"], [BL, H, N], F32,
                          kind="ExternalOutput")

    with tile.TileContext(nc) as tc:
        with (
            tc.tile_pool(name="const", bufs=1) as constp,
            tc.tile_pool(name="at", bufs=2 * NT) as atp,
            tc.tile_pool(name="x", bufs=2) as xp,
            tc.tile_pool(name="h", bufs=2) as hp,
            tc.tile_pool(name="act", bufs=1) as actp,
            tc.tile_pool(name="hop_ps", bufs=4, space=bass.MemorySpace.PSUM) as hop_ps,
            tc.tile_pool(name="tr_ps", bufs=2, space=bass.MemorySpace.PSUM) as tr_ps,
            tc.tile_pool(name="lin_ps", bufs=2, space=bass.MemorySpace.PSUM) as lin_ps,
        ):
            ident = constp.tile([128, 128], F32)
            make_identity(nc, ident[:])

            wru_s = [constp.tile([CC, 2 * H], F32R, tag=f"wru{k}",
                                 name=f"wru_s{k}") for k in W_NAMES]
            wc_s = [constp.tile([CC, H], F32R, tag=f"wc{k}",
                                name=f"wc_s{k}") for k in W_NAMES]
            brus = constp.tile([2 * H, 1], F32, tag="bru")
            bcs = constp.tile([H, 1], F32, tag="bc")
            # constants go via the ACT-triggered HWDGE queue so the
            # SP queue stays exclusively on the A^T stream
            for d, s_ in zip(wru_d, wru_s):
                nc.scalar.dma_start(s_[:], d[:])
            for d, s_ in zip(wc_d, wc_s):
                nc.scalar.dma_start(s_[:], d[:])
            nc.scalar.dma_start(brus[:], bru[:].unsqueeze(1))
            nc.scalar.dma_start(bcs[:], bc[:].unsqueeze(1))

            def hop_stage(x0, x1_, at0, at1, ps):
                """One diffusion hop for both supports.
                x0/x1_: [128, NT, CC] natural lhsT tiles
                at0/at1: lists of NT chunk tiles [128, N]
                ps: dict (s, j) -> PSUM tile [CC, 512] = Y_s^T half."""
                for t in range(NT):
                    for s_, xt in ((0, x0), (1, x1_)):
                        lt = xt[:, t, :]
                        at = at0[t] if s_ == 0 else at1[t]
                        for j in range(2):
                            nc.tensor.matmul(
                                ps[s_, j][:],
                                lt, at[:, j * 512:(j + 1) * 512],
                                start=(t == 0), stop=(t == NT - 1),
                                skip_group_check=True)

            def to_natural(hT, nat):
                """Transpose hT[0:CC, :] ([CC, N]) into natural
                [128, NT, CC] tile nat."""
                ps = tr_ps.tile([128, NT * CC], F32, tag="tr")
                for t in range(NT):
                    nc.tensor.transpose(
                        ps[:, t * CC:(t + 1) * CC],
                        hT[0:CC, t * 128:(t + 1) * 128].bitcast(F32),
                        ident[0:CC, 0:CC],
                    )
                nc.vector.tensor_copy(
                    nat[:, :, :],
                    ps[:].rearrange("p (t c) -> p t c", c=CC))

            for b in range(BL):
                # natural x1 = [inputs | states]
                x1 = xp.tile([128, NT, CC], F32R, tag="x1")
                nc.scalar.dma_start(
                    x1[:, :, 0:CIN],
                    inp[b].rearrange("(t p) c -> p t c", p=128))
                nc.scalar.dma_start(
                    x1[:, :, CIN:CC],
                    st[b].rearrange("(t p) c -> p t c", p=128))

                # A^T chunk tiles (one per m-block, 512 KB each) so hop
                # matmuls start as soon as chunk t=0 lands
                sup_r0 = supT[b, 0].rearrange("(t p) m -> p t m", p=128)
                sup_r1 = supT[b, 1].rearrange("(t p) m -> p t m", p=128)
                at0, at1 = [], []
                for t in range(NT):
                    a0 = atp.tile([128, N], F32R, tag="at0", name=f"at0_{t}")
                    a1 = atp.tile([128, N], F32R, tag="at1", name=f"at1_{t}")
                    nc.sync.dma_start(a0[:], sup_r0[:, t, :])
                    nc.sync.dma_start(a1[:], sup_r1[:, t, :])
                    at0.append(a0)
                    at1.append(a1)

                # states^T for gate & r*state
                stT_sb = actp.tile([H, N], F32R, tag="stT")
                nc.scalar.dma_start(stT_sb[:], stT[b])

                ruT = actp.tile([2 * H, N], F32, tag="ruT")
                cT = actp.tile([H, N], F32, tag="cT")

                for conv in range(2):
                    # ---- hT0 = x^T: rows 0:2 inputs^T, 2:34 (r*)states^T
                    hT0 = hp.tile([CC, N], F32R, tag="hT0")
                    nc.scalar.dma_start(hT0[0:CIN, :], inpT[b])
                    if conv == 0:
                        nc.scalar.dma_start(hT0[CIN:CC, :], stT[b])
                        xn = x1
                    else:
                        # r^T * states^T (computed base-0, partition-shifted
                        # into hT0 rows 2:34 via SBUF->SBUF DMA)
                        rstT = actp.tile([H, N], F32R, tag="scr")
                        nc.vector.tensor_tensor(
                            out=rstT[:], in0=ruT[0:H, :],
                            in1=stT_sb[:].bitcast(F32), op=OP.mult)
                        nc.scalar.dma_start(hT0[CIN:CC, :], rstT[:])
                        # natural x2: transpose r^T, then r*state
                        psr = tr_ps.tile([128, NT * H], F32, tag="tr")
                        for t in range(NT):
                            nc.tensor.transpose(
                                psr[:, t * H:(t + 1) * H],
                                ruT[0:H, t * 128:(t + 1) * 128],
                                ident[0:H, 0:H])
                        x2 = xp.tile([128, NT, CC], F32R, tag="x2")
                        nc.scalar.dma_start(
                            x2[:, :, 0:CIN],
                            inp[b].rearrange("(t p) c -> p t c", p=128))
                        nc.vector.tensor_tensor(
                            out=x2[:, :, CIN:CC],
                            in0=psr[:].rearrange("p (t c) -> p t c", c=H),
                            in1=x1[:, :, CIN:CC].bitcast(F32), op=OP.mult)
                        xn = x2

                    # ---- hop 1 (both supports) ----
                    h1ps = {(s_, j): hop_ps.tile([CC, 512], F32, tag="hop",
                                                 name=f"h1ps{s_}{j}")
                            for s_ in range(2) for j in range(2)}
                    hop_stage(xn, xn, at0, at1, h1ps)
                    hT1a = hp.tile([CC, N], F32R, tag="hT1a")
                    hT1b = hp.tile([CC, N], F32R, tag="hT1b")
                    for j in range(2):
                        sl = slice(j * 512, (j + 1) * 512)
                        nc.vector.tensor_copy(hT1a[:, sl], h1ps[0, j][:])
                        nc.vector.tensor_copy(hT1b[:, sl], h1ps[1, j][:])

                    # ---- naturals of hop-1 outputs ----
                    y0 = xp.tile([128, NT, CC], F32R, tag="y0")
                    y1 = xp.tile([128, NT, CC], F32R, tag="y1")
                    to_natural(hT1a, y0)
                    to_natural(hT1b, y1)

                    # ---- hop 2 ----
                    h2ps = {(s_, j): hop_ps.tile([CC, 512], F32, tag="hop",
                                                 name=f"h2ps{s_}{j}")
                            for s_ in range(2) for j in range(2)}
                    hop_stage(y0, y1, at0, at1, h2ps)
                    hT2a = hp.tile([CC, N], F32R, tag="hT2a")
                    hT2b = hp.tile([CC, N], F32R, tag="hT2b")
                    for j in range(2):
                        sl = slice(j * 512, (j + 1) * 512)
                        nc.vector.tensor_copy(hT2a[:, sl], h2ps[0, j][:])
                        nc.vector.tensor_copy(hT2b[:, sl], h2ps[1, j][:])

                    # ---- linear + activation ----
                    if conv == 0:
                        ws, Mo, bias, dest, func = \
                            wru_s, 2 * H, brus, ruT, AF.Sigmoid
                    else:
                        ws, Mo, bias, dest, func = wc_s, H, bcs, cT, AF.Tanh
                    rhs_tiles = [hT0, hT1a, hT1b, hT2a, hT2b]
                    for j in range(2):
                        pl = lin_ps.tile([2 * H, 512], F32, tag="lin")
                        sl = slice(j * 512, (j + 1) * 512)
                        for k in range(5):
                            nc.tensor.matmul(pl[0:Mo, :], ws[k][:],
                                             rhs_tiles[k][:, sl],
                                             start=(k == 0), stop=(k == 4))
                        nc.scalar.activation(dest[:, sl], pl[0:Mo, :],
                                             func, bias=bias[:])

                # ---- gate: out^T = u^T*(st^T - c^T) + c^T ----
                # u^T rebased to partition 0 via SBUF->SBUF DMA
                uT = actp.tile([H, N], F32, tag="uT")
                nc.scalar.dma_start(uT[:], ruT[H:2 * H, :])
                oT = actp.tile([H, N], F32, tag="oT")
                nc.vector.tensor_tensor(out=oT[:], in0=stT_sb[:].bitcast(F32),
                                        in1=cT[:], op=OP.subtract)
                nc.vector.tensor_tensor(out=oT[:], in0=uT[:],
                                        in1=oT[:], op=OP.mult)
                nc.vector.tensor_tensor(out=oT[:], in0=oT[:],
                                        in1=cT[:], op=OP.add)
                nc.scalar.dma_start(outT[b], oT[:])

    nc.compile()
    return nc


def _split_w(W):
    """Split W [170, cols] into the 5 feature blocks in h^T tile order:
    x, A0x, A1x, A0^2 x, A1^2 x."""
    blocks = [W[0:CC]]
    for fidx in (1, 3, 2, 4):
        blocks.append(W[fidx * CC:(fidx + 1) * CC])
    return [np.ascontiguousarray(w, np.float32) for w in blocks]


def kernel(inputs, supports, states, W_ru, b_ru, W_c, b_c, **_ignored):
    global LAST_RESULTS
    inputs = np.ascontiguousarray(np.asarray(inputs, np.float32))
    supports = np.asarray(supports, np.float32)
    states = np.ascontiguousarray(np.asarray(states, np.float32))
    W_ru = np.asarray(W_ru, np.float32)
    b_ru = np.ascontiguousarray(np.asarray(b_ru, np.float32))
    W_c = np.asarray(W_c, np.float32)
    b_c = np.ascontiguousarray(np.asarray(b_c, np.float32))

    supT = np.ascontiguousarray(np.swapaxes(supports, 2, 3))
    inpT = np.ascontiguousarray(np.swapaxes(inputs, 1, 2))
    stT = np.ascontiguousarray(np.swapaxes(states, 1, 2))
    wru_blocks = _split_w(W_ru)
    wc_blocks = _split_w(W_c)

    if "nc" not in _CACHE:
        _CACHE["nc"] = _build()
    nc = _CACHE["nc"]

    in_maps = []
    for i in range(NCORES):
        sl = slice(i * BL, (i + 1) * BL)
        m = {
            "inp": inputs[sl], "inpT": inpT[sl],
            "st": states[sl], "stT": stT[sl], "supT": supT[sl],
            "bru": b_ru, "bc": b_c,
        }
        for k, wb in zip(W_NAMES, wru_blocks):
            m[f"wru_{k}"] = wb
        for k, wb in zip(W_NAMES, wc_blocks):
            m[f"wc_{k}"] = wb
        in_maps.append(m)

    res = run_bass_kernel_spmd(nc, in_maps, list(range(NCORES)))
    LAST_RESULTS = res
    oname = _CACHE["out_name"]
    out = np.concatenate(
        [np.swapaxes(res.results[i][oname], 1, 2) for i in range(NCORES)],
        axis=0)
    return (out, out)


# revision 19
# speedup vs baseline: 1.0111x; 1.0019x over previous
"""DCGRU cell (dense diffusion graph conv GRU) on 8 Trainium2 NeuronCores.

Strategy (data-parallel over batch, 4 batch elements per core):
  - supports are pre-transposed on host (A^T, contiguous) so the tensor
    engine can contract over the node dimension m with m on partitions.
  - node-feature matrices that feed the final linear are kept in
    TRANSPOSED form h^T [features, nodes]; hop outputs are produced
    transposed directly:  Y^T = matmul(lhsT=x_chunk [m,34], rhs=A^T [m,n]).
  - the final linear is 5 accumulating matmuls, one per 34-row feature
    block (weights sliced host-side to match).
  - matmuls run as float32r (fp32 storage, single-pass reduced-precision
    PE mode, 1 cycle/row at free-dim 512; PSUM dst must start at
    partition 0); transposes stay exact fp32.
  - gate math is done in transposed space; the output is produced as
    out^T [H, N] and un-transposed on host.
"""

import hashlib
import inspect

import numpy as np

import concourse.bacc as bacc
import concourse.mybir as mybir
import concourse.tile as tile
import concourse.bass as bass
from concourse.bass_utils import run_bass_kernel_spmd
from concourse.masks import make_identity

B, N, CIN, H, S, K = 32, 1024, 2, 32, 2, 2
NCORES = 8
BL = B // NCORES          # batch elements per core
CC = CIN + H              # 34 concat features
NT = N // 128             # 8 node chunks of 128
F32 = mybir.dt.float32
F32R = mybir.dt.float32r
AF = mybir.ActivationFunctionType
OP = mybir.AluOpType

_CACHE = {}
LAST_RESULTS = None

W_NAMES = ["w0", "w1a", "w1b", "w2a", "w2b"]


def _build():
    # Version-tag the output tensor name with a hash of this function's
    # source: the neuron compile cache keys on HLO (which embeds tensor
    # names but not the BIR), so kernel-internal changes need a name
    # change to invalidate stale NEFFs.
    vtag = hashlib.md5(inspect.getsource(_build).encode()).hexdigest()[:8]
    _CACHE["out_name"] = f"outT_{vtag}"
    nc = bacc.Bacc("TRN2", target_bir_lowering=False, debug=False,
                   num_devices=NCORES)

    inp = nc.dram_tensor("inp", [BL, N, CIN], F32R, kind="ExternalInput")
    inpT = nc.dram_tensor("inpT", [BL, CIN, N], F32R, kind="ExternalInput")
    st = nc.dram_tensor("st", [BL, N, H], F32R, kind="ExternalInput")
    stT = nc.dram_tensor("stT", [BL, H, N], F32R, kind="ExternalInput")
    supT = nc.dram_tensor("supT", [BL, S, N, N], F32R, kind="ExternalInput")
    wru_d = [nc.dram_tensor(f"wru_{k}", [CC, 2 * H], F32R,
                            kind="ExternalInput") for k in W_NAMES]
    wc_d = [nc.dram_tensor(f"wc_{k}", [CC, H], F32R,
                           kind="ExternalInput") for k in W_NAMES]
    bru = nc.dram_tensor("bru", [2 * H], F32, kind="ExternalInput")
    bc = nc.dram_tensor("bc", [H], F32, kind="ExternalInput")
    outT = nc.dram_tensor(_CACHE["out_name"], [BL, H, N], F32,
                          kind="ExternalOutput")

    with tile.TileContext(nc) as tc:
        with (
            tc.tile_pool(name="const", bufs=1) as constp,
            tc.tile_pool(name="at", bufs=4) as atp,
            tc.tile_pool(name="x", bufs=2) as xp,
            tc.tile_pool(name="h", bufs=2) as hp,
            tc.tile_pool(name="act", bufs=1) as actp,
            tc.tile_pool(name="hop_ps", bufs=5, space=bass.MemorySpace.PSUM) as hop_ps,
            tc.tile_pool(name="aux_ps", bufs=3, space=bass.MemorySpace.PSUM) as aux_ps,
        ):
            ident = constp.tile([128, 128], F32)
            make_identity(nc, ident[:])

            wru_s = [constp.tile([CC, 2 * H], F32R, tag=f"wru{k}",
                                 name=f"wru_s{k}") for k in W_NAMES]
            wc_s = [constp.tile([CC, H], F32R, tag=f"wc{k}",
                                name=f"wc_s{k}") for k in W_NAMES]
            brus = constp.tile([2 * H, 1], F32, tag="bru")
            bcs = constp.tile([H, 1], F32, tag="bc")
            # constants go via the ACT-triggered HWDGE queue so the
            # SP queue stays exclusively on the A^T stream
            for d, s_ in zip(wru_d, wru_s):
                nc.scalar.dma_start(s_[:], d[:])
            for d, s_ in zip(wc_d, wc_s):
                nc.scalar.dma_start(s_[:], d[:])
            nc.scalar.dma_start(brus[:], bru[:].unsqueeze(1))
            nc.scalar.dma_start(bcs[:], bc[:].unsqueeze(1))

            def hop_stage(x0, x1_, at0, at1, ps):
                """One diffusion hop for both supports, s-major so each
                support's PSUM banks complete (and free) early.
                x0/x1_: [128, NT, CC] natural lhsT tiles
                at0/at1: lists of 2 super-chunk tiles [128, NT//2, N]
                ps: dict (s, j) -> PSUM tile [CC, 512] = Y_s^T half."""
                half = NT // 2
                for s_, xt in ((0, x0), (1, x1_)):
                    ats = at0 if s_ == 0 else at1
                    for t in range(NT):
                        lt = xt[:, t, :]
                        at = ats[t // half][:, t % half, :]
                        for j in range(2):
                            nc.tensor.matmul(
                                ps[s_, j][:],
                                lt, at[:, j * 512:(j + 1) * 512],
                                start=(t == 0), stop=(t == NT - 1),
                                skip_group_check=True)

            def to_natural(hT, nat):
                """Transpose hT[0:CC, :] ([CC, N]) into natural
                [128, NT, CC] tile nat."""
                ps = aux_ps.tile([128, NT * CC], F32, tag="aux")
                for t in range(NT):
                    nc.tensor.transpose(
                        ps[:, t * CC:(t + 1) * CC],
                        hT[0:CC, t * 128:(t + 1) * 128].bitcast(F32),
                        ident[0:CC, 0:CC],
                    )
                nc.vector.tensor_copy(
                    nat[:, :, :],
                    ps[:].rearrange("p (t c) -> p t c", c=CC))

            for b in range(BL):
                # natural x1 = [inputs | states]
                x1 = xp.tile([128, NT, CC], F32R, tag="x1")
                nc.scalar.dma_start(
                    x1[:, :, 0:CIN],
                    inp[b].rearrange("(t p) c -> p t c", p=128))
                nc.scalar.dma_start(
                    x1[:, :, CIN:CC],
                    st[b].rearrange("(t p) c -> p t c", p=128))

                # A^T super-chunk tiles (4 m-blocks / 2 MB each): large
                # enough to amortize the ~0.9us per-DMA queue overhead,
                # small enough that the first hop matmuls start early.
                # s=0 goes through the SP HWDGE trigger, s=1 through ACT,
                # so the two streams ride different queues.
                half = NT // 2
                sup_r0 = supT[b, 0].rearrange("(t p) m -> p t m", p=128)
                sup_r1 = supT[b, 1].rearrange("(t p) m -> p t m", p=128)
                at0, at1 = [], []
                for hh in range(2):
                    a0 = atp.tile([128, half, N], F32R, tag="at0",
                                  name=f"at0_{hh}")
                    a1 = atp.tile([128, half, N], F32R, tag="at1",
                                  name=f"at1_{hh}")
                    nc.sync.dma_start(
                        a0[:], sup_r0[:, hh * half:(hh + 1) * half, :])
                    nc.scalar.dma_start(
                        a1[:], sup_r1[:, hh * half:(hh + 1) * half, :])
                    at0.append(a0)
                    at1.append(a1)

                # states^T for gate & r*state
                stT_sb = actp.tile([H, N], F32R, tag="stT")
                nc.scalar.dma_start(stT_sb[:], stT[b])

                ruT = actp.tile([2 * H, N], F32, tag="ruT")
                cT = actp.tile([H, N], F32, tag="cT")

                for conv in range(2):
                    # ---- hT0 = x^T: rows 0:2 inputs^T, 2:34 (r*)states^T
                    hT0 = hp.tile([CC, N], F32R, tag="hT0")
                    nc.scalar.dma_start(hT0[0:CIN, :], inpT[b])
                    if conv == 0:
                        nc.scalar.dma_start(hT0[CIN:CC, :], stT[b])
                        xn = x1
                    else:
                        # r^T * states^T (computed base-0, partition-shifted
                        # into hT0 rows 2:34 via SBUF->SBUF DMA)
                        rstT = actp.tile([H, N], F32R, tag="scr")
                        nc.vector.tensor_tensor(
                            out=rstT[:], in0=ruT[0:H, :],
                            in1=stT_sb[:].bitcast(F32), op=OP.mult)
                        nc.scalar.dma_start(hT0[CIN:CC, :], rstT[:])
                        # natural x2: transpose r^T, then r*state
                        psr = aux_ps.tile([128, NT * H], F32, tag="aux")
                        for t in range(NT):
                            nc.tensor.transpose(
                                psr[:, t * H:(t + 1) * H],
                                ruT[0:H, t * 128:(t + 1) * 128],
                                ident[0:H, 0:H])
                        x2 = xp.tile([128, NT, CC], F32R, tag="x2")
                        nc.scalar.dma_start(
                            x2[:, :, 0:CIN],
                            inp[b].rearrange("(t p) c -> p t c", p=128))
                        nc.vector.tensor_tensor(
                            out=x2[:, :, CIN:CC],
                            in0=psr[:].rearrange("p (t c) -> p t c", c=H),
                            in1=x1[:, :, CIN:CC].bitcast(F32), op=OP.mult)
                        xn = x2

                    # ---- hop 1 (both supports) ----
                    h1ps = {(s_, j): hop_ps.tile([CC, 512], F32, tag="hop",
                                                 name=f"h1ps{s_}{j}")
                            for s_ in range(2) for j in range(2)}
                    hop_stage(xn, xn, at0, at1, h1ps)
                    hT1a = hp.tile([CC, N], F32R, tag="hT1a")
                    hT1b = hp.tile([CC, N], F32R, tag="hT1b")
                    for j in range(2):
                        sl = slice(j * 512, (j + 1) * 512)
                        nc.vector.tensor_copy(hT1a[:, sl], h1ps[0, j][:])
                        nc.vector.tensor_copy(hT1b[:, sl], h1ps[1, j][:])

                    # ---- naturals of hop-1 outputs ----
                    y0 = xp.tile([128, NT, CC], F32R, tag="y0")
                    y1 = xp.tile([128, NT, CC], F32R, tag="y1")
                    to_natural(hT1a, y0)
                    to_natural(hT1b, y1)

                    # ---- hop 2 ----
                    h2ps = {(s_, j): hop_ps.tile([CC, 512], F32, tag="hop",
                                                 name=f"h2ps{s_}{j}")
                            for s_ in range(2) for j in range(2)}
                    hop_stage(y0, y1, at0, at1, h2ps)
                    hT2a = hp.tile([CC, N], F32R, tag="hT2a")
                    hT2b = hp.tile([CC, N], F32R, tag="hT2b")
                    for j in range(2):
                        sl = slice(j * 512, (j + 1) * 512)
                        nc.vector.tensor_copy(hT2a[:, sl], h2ps[0, j][:])
                        nc.vector.tensor_copy(hT2b[:, sl], h2ps[1, j][:])

                    # ---- linear + activation ----
                    if conv == 0:
                        ws, Mo, bias, dest, func = \
                            wru_s, 2 * H, brus, ruT, AF.Sigmoid
                    else:
                        ws, Mo, bias, dest, func = wc_s, H, bcs, cT, AF.Tanh
                    rhs_tiles = [hT0, hT1a, hT1b, hT2a, hT2b]
                    for j in range(2):
                        pl = aux_ps.tile([2 * H, 512], F32, tag="aux")
                        sl = slice(j * 512, (j + 1) * 512)
                        for k in range(5):
                            nc.tensor.matmul(pl[0:Mo, :], ws[k][:],
                                             rhs_tiles[k][:, sl],
                                             start=(k == 0), stop=(k == 4))
                        nc.scalar.activation(dest[:, sl], pl[0:Mo, :],
                                             func, bias=bias[:])

                # ---- gate: out^T = u^T*(st^T - c^T) + c^T ----
                # u^T rebased to partition 0 via SBUF->SBUF DMA
                uT = actp.tile([H, N], F32, tag="uT")
                nc.scalar.dma_start(uT[:], ruT[H:2 * H, :])
                oT = actp.tile([H, N], F32, tag="oT")
                nc.vector.tensor_tensor(out=oT[:], in0=stT_sb[:].bitcast(F32),
                                        in1=cT[:], op=OP.subtract)
                nc.vector.tensor_tensor(out=oT[:], in0=uT[:],
                                        in1=oT[:], op=OP.mult)
                nc.vector.tensor_tensor(out=oT[:], in0=oT[:],
                                        in1=cT[:], op=OP.add)
                nc.scalar.dma_start(outT[b], oT[:])

    nc.compile()
    return nc


def _split_w(W):
    """Split W [170, cols] into the 5 feature blocks in h^T tile order:
    x, A0x, A1x, A0^2 x, A1^2 x."""
    blocks = [W[0:CC]]
    for fidx in (1, 3, 2, 4):
        blocks.append(W[fidx * CC:(fidx + 1) * CC])
    return [np.ascontiguousarray(w, np.float32) for w in blocks]


def kernel(inputs, supports, states, W_ru, b_ru, W_c, b_c, **_ignored):
    global LAST_RESULTS
    inputs = np.ascontiguousarray(np.asarray(inputs, np.float32))
    supports = np.asarray(supports, np.float32)
    states = np.ascontiguousarray(np.asarray(states, np.float32))
    W_ru = np.asarray(W_ru, np.float32)
    b_ru = np.ascontiguousarray(np.asarray(b_ru, np.float32))
    W_c = np.asarray(W_c, np.float32)
    b_c = np.ascontiguousarray(np.asarray(b_c, np.float32))

    supT = np.ascontiguousarray(np.swapaxes(supports, 2, 3))
    inpT = np.ascontiguousarray(np.swapaxes(inputs, 1, 2))
    stT = np.ascontiguousarray(np.swapaxes(states, 1, 2))
    wru_blocks = _split_w(W_ru)
    wc_blocks = _split_w(W_c)

    if "nc" not in _CACHE:
        _CACHE["nc"] = _build()
    nc = _CACHE["nc"]

    in_maps = []
    for i in range(NCORES):
        sl = slice(i * BL, (i + 1) * BL)
        m = {
            "inp": inputs[sl], "inpT": inpT[sl],
            "st": states[sl], "stT": stT[sl], "supT": supT[sl],
            "bru": b_ru, "bc": b_c,
        }
        for k, wb in zip(W_NAMES, wru_blocks):
            m[f"wru_{k}"] = wb
        for k, wb in zip(W_NAMES, wc_blocks):
            m[f"wc_{k}"] = wb
        in_maps.append(m)

    res = run_bass_kernel_spmd(nc, in_maps, list(range(NCORES)))
    LAST_RESULTS = res
    oname = _CACHE["out_name"]
    out = np.concatenate(
        [np.swapaxes(res.results[i][oname], 1, 2) for i in range(NCORES)],
        axis=0)
    return (out, out)


# revision 20
# speedup vs baseline: 1.0770x; 1.0652x over previous
"""DCGRU cell (dense diffusion graph conv GRU) on 8 Trainium2 NeuronCores.

Strategy (data-parallel over batch, 4 batch elements per core):
  - supports are pre-transposed on host (A^T, contiguous) so the tensor
    engine can contract over the node dimension m with m on partitions.
  - node-feature matrices that feed the final linear are kept in
    TRANSPOSED form h^T [features, nodes]; hop outputs are produced
    transposed directly:  Y^T = matmul(lhsT=x_chunk [m,34], rhs=A^T [m,n]).
  - the final linear is 5 accumulating matmuls, one per 34-row feature
    block (weights sliced host-side to match).
  - matmuls run as float32r (fp32 storage, single-pass reduced-precision
    PE mode, 1 cycle/row at free-dim 512; PSUM dst must start at
    partition 0); transposes stay exact fp32.
  - gate math is done in transposed space; the output is produced as
    out^T [H, N] and un-transposed on host.
"""

import hashlib
import inspect

import numpy as np

import concourse.bacc as bacc
import concourse.mybir as mybir
import concourse.tile as tile
import concourse.bass as bass
from concourse.bass_utils import run_bass_kernel_spmd
from concourse.masks import make_identity

B, N, CIN, H, S, K = 32, 1024, 2, 32, 2, 2
NCORES = 8
BL = B // NCORES          # batch elements per core
CC = CIN + H              # 34 concat features
NT = N // 128             # 8 node chunks of 128
F32 = mybir.dt.float32
F32R = mybir.dt.float32r
AF = mybir.ActivationFunctionType
OP = mybir.AluOpType

_CACHE = {}
LAST_RESULTS = None

W_NAMES = ["w0", "w1a", "w1b", "w2a", "w2b"]


def _build():
    # Version-tag the output tensor name with a hash of this function's
    # source: the neuron compile cache keys on HLO (which embeds tensor
    # names but not the BIR), so kernel-internal changes need a name
    # change to invalidate stale NEFFs.
    vtag = hashlib.md5(inspect.getsource(_build).encode()).hexdigest()[:8]
    _CACHE["out_name"] = f"outT_{vtag}"
    nc = bacc.Bacc("TRN2", target_bir_lowering=False, debug=False,
                   num_devices=NCORES)

    inp = nc.dram_tensor("inp", [BL, N, CIN], F32R, kind="ExternalInput")
    inpT = nc.dram_tensor("inpT", [BL, CIN, N], F32R, kind="ExternalInput")
    st = nc.dram_tensor("st", [BL, N, H], F32R, kind="ExternalInput")
    stT = nc.dram_tensor("stT", [BL, H, N], F32R, kind="ExternalInput")
    supT = nc.dram_tensor("supT", [BL, S, N, N], F32R, kind="ExternalInput")
    wru_d = [nc.dram_tensor(f"wru_{k}", [CC, 2 * H], F32R,
                            kind="ExternalInput") for k in W_NAMES]
    wc_d = [nc.dram_tensor(f"wc_{k}", [CC, H], F32R,
                           kind="ExternalInput") for k in W_NAMES]
    bru = nc.dram_tensor("bru", [2 * H], F32, kind="ExternalInput")
    bc = nc.dram_tensor("bc", [H], F32, kind="ExternalInput")
    outT = nc.dram_tensor(_CACHE["out_name"], [BL, H, N], F32,
                          kind="ExternalOutput")

    with tile.TileContext(nc) as tc:
        with (
            tc.tile_pool(name="const", bufs=1) as constp,
            tc.tile_pool(name="at", bufs=4) as atp,
            tc.tile_pool(name="x", bufs=2) as xp,
            tc.tile_pool(name="h", bufs=2) as hp,
            tc.tile_pool(name="act", bufs=1) as actp,
            tc.tile_pool(name="hop_ps", bufs=5, space=bass.MemorySpace.PSUM) as hop_ps,
            tc.tile_pool(name="aux_ps", bufs=3, space=bass.MemorySpace.PSUM) as aux_ps,
        ):
            ident = constp.tile([128, 128], F32)
            make_identity(nc, ident[:])

            wru_s = [constp.tile([CC, 2 * H], F32R, tag=f"wru{k}",
                                 name=f"wru_s{k}") for k in W_NAMES]
            wc_s = [constp.tile([CC, H], F32R, tag=f"wc{k}",
                                name=f"wc_s{k}") for k in W_NAMES]
            brus = constp.tile([2 * H, 1], F32, tag="bru")
            bcs = constp.tile([H, 1], F32, tag="bc")
            # constants go via the ACT-triggered HWDGE queue so the
            # SP queue stays exclusively on the A^T stream
            for d, s_ in zip(wru_d, wru_s):
                nc.scalar.dma_start(s_[:], d[:])
            for d, s_ in zip(wc_d, wc_s):
                nc.scalar.dma_start(s_[:], d[:])
            nc.scalar.dma_start(brus[:], bru[:].unsqueeze(1))
            nc.scalar.dma_start(bcs[:], bc[:].unsqueeze(1))

            def hop_stage(x0, x1_, at0, at1, ps):
                """One diffusion hop for both supports, s-major so each
                support's PSUM banks complete (and free) early.
                x0/x1_: [128, NT, CC] natural lhsT tiles
                at0/at1: lists of 2 super-chunk tiles [128, NT//2, N]
                ps: dict (s, j) -> PSUM tile [CC, 512] = Y_s^T half."""
                half = NT // 2
                for s_, xt in ((0, x0), (1, x1_)):
                    ats = at0 if s_ == 0 else at1
                    for t in range(NT):
                        lt = xt[:, t, :]
                        at = ats[t // half][:, t % half, :]
                        for j in range(2):
                            nc.tensor.matmul(
                                ps[s_, j][:],
                                lt, at[:, j * 512:(j + 1) * 512],
                                start=(t == 0), stop=(t == NT - 1),
                                skip_group_check=True)

            def to_natural(hT, nat):
                """Transpose hT[0:CC, :] ([CC, N]) into natural
                [128, NT, CC] tile nat."""
                ps = aux_ps.tile([128, NT * CC], F32, tag="aux")
                for t in range(NT):
                    nc.tensor.transpose(
                        ps[:, t * CC:(t + 1) * CC],
                        hT[0:CC, t * 128:(t + 1) * 128].bitcast(F32),
                        ident[0:CC, 0:CC],
                    )
                nc.vector.tensor_copy(
                    nat[:, :, :],
                    ps[:].rearrange("p (t c) -> p t c", c=CC))

            for b in range(BL):
                # natural x1 = [inputs | states]
                x1 = xp.tile([128, NT, CC], F32R, tag="x1")
                nc.gpsimd.dma_start(
                    x1[:, :, 0:CIN],
                    inp[b].rearrange("(t p) c -> p t c", p=128))
                nc.gpsimd.dma_start(
                    x1[:, :, CIN:CC],
                    st[b].rearrange("(t p) c -> p t c", p=128))

                # A^T super-chunk tiles (4 m-blocks / 2 MB each): large
                # enough to amortize the ~0.9us per-DMA queue overhead,
                # small enough that the first hop matmuls start early.
                # s=0 goes through the SP HWDGE trigger, s=1 through ACT,
                # so the two streams ride different queues.
                half = NT // 2
                sup_r0 = supT[b, 0].rearrange("(t p) m -> p t m", p=128)
                sup_r1 = supT[b, 1].rearrange("(t p) m -> p t m", p=128)
                at0, at1 = [], []
                for hh in range(2):
                    a0 = atp.tile([128, half, N], F32R, tag="at0",
                                  name=f"at0_{hh}")
                    a1 = atp.tile([128, half, N], F32R, tag="at1",
                                  name=f"at1_{hh}")
                    nc.sync.dma_start(
                        a0[:], sup_r0[:, hh * half:(hh + 1) * half, :])
                    nc.sync.dma_start(
                        a1[:], sup_r1[:, hh * half:(hh + 1) * half, :])
                    at0.append(a0)
                    at1.append(a1)

                # states^T for gate & r*state
                stT_sb = actp.tile([H, N], F32R, tag="stT")
                nc.gpsimd.dma_start(stT_sb[:], stT[b])

                ruT = actp.tile([2 * H, N], F32, tag="ruT")
                cT = actp.tile([H, N], F32, tag="cT")

                for conv in range(2):
                    # ---- hT0 = x^T: rows 0:2 inputs^T, 2:34 (r*)states^T
                    hT0 = hp.tile([CC, N], F32R, tag="hT0")
                    nc.gpsimd.dma_start(hT0[0:CIN, :], inpT[b])
                    if conv == 0:
                        nc.gpsimd.dma_start(hT0[CIN:CC, :], stT[b])
                        xn = x1
                    else:
                        # r^T * states^T (computed base-0, partition-shifted
                        # into hT0 rows 2:34 via SBUF->SBUF DMA)
                        rstT = actp.tile([H, N], F32R, tag="scr")
                        nc.vector.tensor_tensor(
                            out=rstT[:], in0=ruT[0:H, :],
                            in1=stT_sb[:].bitcast(F32), op=OP.mult)
                        nc.scalar.dma_start(hT0[CIN:CC, :], rstT[:])
                        # natural x2: transpose r^T, then r*state
                        psr = aux_ps.tile([128, NT * H], F32, tag="aux")
                        for t in range(NT):
                            nc.tensor.transpose(
                                psr[:, t * H:(t + 1) * H],
                                ruT[0:H, t * 128:(t + 1) * 128],
                                ident[0:H, 0:H])
                        x2 = xp.tile([128, NT, CC], F32R, tag="x2")
                        nc.gpsimd.dma_start(
                            x2[:, :, 0:CIN],
                            inp[b].rearrange("(t p) c -> p t c", p=128))
                        nc.vector.tensor_tensor(
                            out=x2[:, :, CIN:CC],
                            in0=psr[:].rearrange("p (t c) -> p t c", c=H),
                            in1=x1[:, :, CIN:CC].bitcast(F32), op=OP.mult)
                        xn = x2

                    # ---- hop 1 (both supports) ----
                    h1ps = {(s_, j): hop_ps.tile([CC, 512], F32, tag="hop",
                                                 name=f"h1ps{s_}{j}")
                            for s_ in range(2) for j in range(2)}
                    hop_stage(xn, xn, at0, at1, h1ps)
                    hT1a = hp.tile([CC, N], F32R, tag="hT1a")
                    hT1b = hp.tile([CC, N], F32R, tag="hT1b")
                    for j in range(2):
                        sl = slice(j * 512, (j + 1) * 512)
                        nc.vector.tensor_copy(hT1a[:, sl], h1ps[0, j][:])
                        nc.vector.tensor_copy(hT1b[:, sl], h1ps[1, j][:])

                    # ---- naturals of hop-1 outputs ----
                    y0 = xp.tile([128, NT, CC], F32R, tag="y0")
                    y1 = xp.tile([128, NT, CC], F32R, tag="y1")
                    to_natural(hT1a, y0)
                    to_natural(hT1b, y1)

                    # ---- hop 2 ----
                    h2ps = {(s_, j): hop_ps.tile([CC, 512], F32, tag="hop",
                                                 name=f"h2ps{s_}{j}")
                            for s_ in range(2) for j in range(2)}
                    hop_stage(y0, y1, at0, at1, h2ps)
                    hT2a = hp.tile([CC, N], F32R, tag="hT2a")
                    hT2b = hp.tile([CC, N], F32R, tag="hT2b")
                    for j in range(2):
                        sl = slice(j * 512, (j + 1) * 512)
                        nc.vector.tensor_copy(hT2a[:, sl], h2ps[0, j][:])
                        nc.vector.tensor_copy(hT2b[:, sl], h2ps[1, j][:])

                    # ---- linear + activation ----
                    if conv == 0:
                        ws, Mo, bias, dest, func = \
                            wru_s, 2 * H, brus, ruT, AF.Sigmoid
                    else:
                        ws, Mo, bias, dest, func = wc_s, H, bcs, cT, AF.Tanh
                    rhs_tiles = [hT0, hT1a, hT1b, hT2a, hT2b]
                    for j in range(2):
                        pl = aux_ps.tile([2 * H, 512], F32, tag="aux")
                        sl = slice(j * 512, (j + 1) * 512)
                        for k in range(5):
                            nc.tensor.matmul(pl[0:Mo, :], ws[k][:],
                                             rhs_tiles[k][:, sl],
                                             start=(k == 0), stop=(k == 4))
                        nc.scalar.activation(dest[:, sl], pl[0:Mo, :],
                                             func, bias=bias[:])

                # ---- gate: out^T = u^T*(st^T - c^T) + c^T ----
                # u^T rebased to partition 0 via SBUF->SBUF DMA
                uT = actp.tile([H, N], F32, tag="uT")
                nc.scalar.dma_start(uT[:], ruT[H:2 * H, :])
                oT = actp.tile([H, N], F32, tag="oT")
                nc.vector.tensor_tensor(out=oT[:], in0=stT_sb[:].bitcast(F32),
                                        in1=cT[:], op=OP.subtract)
                nc.vector.tensor_tensor(out=oT[:], in0=uT[:],
                                        in1=oT[:], op=OP.mult)
                nc.vector.tensor_tensor(out=oT[:], in0=oT[:],
                                        in1=cT[:], op=OP.add)
                nc.scalar.dma_start(outT[b], oT[:])

    nc.compile()
    return nc


def _split_w(W):
    """Split W [170, cols] into the 5 feature blocks in h^T tile order:
    x, A0x, A1x, A0^2 x, A1^2 x."""
    blocks = [W[0:CC]]
    for fidx in (1, 3, 2, 4):
        blocks.append(W[fidx * CC:(fidx + 1) * CC])
    return [np.ascontiguousarray(w, np.float32) for w in blocks]


def kernel(inputs, supports, states, W_ru, b_ru, W_c, b_c, **_ignored):
    global LAST_RESULTS
    inputs = np.ascontiguousarray(np.asarray(inputs, np.float32))
    supports = np.asarray(supports, np.float32)
    states = np.ascontiguousarray(np.asarray(states, np.float32))
    W_ru = np.asarray(W_ru, np.float32)
    b_ru = np.ascontiguousarray(np.asarray(b_ru, np.float32))
    W_c = np.asarray(W_c, np.float32)
    b_c = np.ascontiguousarray(np.asarray(b_c, np.float32))

    supT = np.ascontiguousarray(np.swapaxes(supports, 2, 3))
    inpT = np.ascontiguousarray(np.swapaxes(inputs, 1, 2))
    stT = np.ascontiguousarray(np.swapaxes(states, 1, 2))
    wru_blocks = _split_w(W_ru)
    wc_blocks = _split_w(W_c)

    if "nc" not in _CACHE:
        _CACHE["nc"] = _build()
    nc = _CACHE["nc"]

    in_maps = []
    for i in range(NCORES):
        sl = slice(i * BL, (i + 1) * BL)
        m = {
            "inp": inputs[sl], "inpT": inpT[sl],
            "st": states[sl], "stT": stT[sl], "supT": supT[sl],
            "bru": b_ru, "bc": b_c,
        }
        for k, wb in zip(W_NAMES, wru_blocks):
            m[f"wru_{k}"] = wb
        for k, wb in zip(W_NAMES, wc_blocks):
            m[f"wc_{k}"] = wb
        in_maps.append(m)

    res = run_bass_kernel_spmd(nc, in_maps, list(range(NCORES)))
    LAST_RESULTS = res
    oname = _CACHE["out_name"]
    out = np.concatenate(
        [np.swapaxes(res.results[i][oname], 1, 2) for i in range(NCORES)],
        axis=0)
    return (out, out)
